# revision 12
# baseline (speedup 1.0000x reference)
"""Trainium2 Bass kernel for the moe_routing classifier problem (v2).

Computation (per batch row b, class c):
  score[b,c,s] = (1 + cos(emb[b], W[c,s]))/2 + 1e-8     (S=64 sub-prototypes)
  top-8 over s, softmax weights w, protos = sum_k w_k * W[c, idx_k]
  out[b,c]     = ((1 + cos(protos, emb[b]))/2 + 1e-8) / 0.1

Algebra (Z = unnormalized softmax sum cancels in the cosine ratio):
  E[b,cs]   = exp(score) masked to the top-8 entries   (threshold mask)
  d2'[b,c]  = sum_s E * (W_s . emb_b/|emb_b|)          (prodD in s-major space)
  np2[b,c]  = |L_c^T E_c^T|^2 = E^T G E                (G = W W^T = L L^T, host chol)
  out       = 5 * d2' / sqrt(np2) + 5 + 1e-7           (1/sqrt via exp(-.5 ln))

Host prep (per core, weights-only): V^T (rows normalized), W^T, block-diag
pair Cholesky factors LP, a block-ones reduction matrix; all bf16.

Device per batch tile (128 rows):
  PE : dot = embT^T V       (b-major, for scores)
       dotT = W embT        (s-major, raw dots for d2')
       F = E^T (16 transposes), M = LP^T F, segmented s-sums of
       [M^2 | F*dotT] via 16 accumulating block-ones matmuls -> psum [32,2,128]
  ACT: exps = Exp(dot*hine+bias) bf16; F/Msq copies; tail ln/exp/out-copy
  DVE: 32x MAX8 (8th-largest per class), E = mask*exps (bf16 2x), tail muls
  POOL: mask = exps >= t8 (broadcast threshold), prodD = dotT * F

Engines never touch Sqrt: all ACT funcs (Exp/Ln/Square/Copy/Identity) live in
the natural_log_exp_and_others table -> zero table reloads.

Sharding: classes split across 8 cores (32 each); emb replicated.
"""

import numpy as np

B, D, C, S = 1024, 128, 256, 64
NCORES = 8
C_LOC = C // NCORES        # 32 classes per core
CS = C_LOC * S             # 2048 anchor rows per core
P = 128                    # partitions
NBT = B // P               # 8 batch tiles
NPAIR = C_LOC // 2         # 16 class pairs (128 anchor rows each)
EPS = 1e-8
SC_BIAS = 0.5 + EPS        # score = 0.5*cos + SC_BIAS
OUT_SCALE = 5.0            # ((1+x)/2 + 1e-8) / 0.1 = 5x + 5 + 1e-7
OUT_BIAS = 5.0 + 1e-7

_CACHE = {}


def _ones32() -> np.ndarray:
    """Block-ones stationary [128, NPAIR, C_LOC]: chunk q reduces partitions
    0:64 -> class 2q, 64:128 -> class 2q+1; all other columns zero so the 16
    matmuls can accumulate into one [C_LOC, ...] psum region."""
    o = np.zeros((P, NPAIR, C_LOC), np.float32)
    for q in range(NPAIR):
        o[0:64, q, 2 * q] = 1.0
        o[64:128, q, 2 * q + 1] = 1.0
    return o


def core_inputs(emb: np.ndarray, weight: np.ndarray, i: int) -> dict:
    """Host-side prep for core i: bf16 cast, V/W transposes, pair-packed
    Cholesky factors of the per-class Gram matrices."""
    import ml_dtypes

    bf = ml_dtypes.bfloat16
    Wc = np.ascontiguousarray(weight[i * C_LOC : (i + 1) * C_LOC]).astype(
        np.float64
    )                                              # [32, 64, 128]
    G = np.einsum("csd,ctd->cst", Wc, Wc)          # [32, 64, 64]
    jit = 1e-9 * np.einsum("css->c", G) / S
    G += jit[:, None, None] * np.eye(S)
    L = np.linalg.cholesky(G)                      # lower: G = L L^T
    lp = np.zeros((P, NPAIR, P), np.float32)
    for q in range(NPAIR):
        lp[0:S, q, 0:S] = L[2 * q]
        lp[S:P, q, S:P] = L[2 * q + 1]
    W2 = Wc.reshape(CS, D)
    nw = np.maximum(np.linalg.norm(W2, axis=1), EPS)
    V2 = W2 / nw[:, None]
    return {
        "emb": emb.astype(bf),
        "vt": np.ascontiguousarray(V2.T.astype(np.float32)).astype(bf),
        "wt": np.ascontiguousarray(W2.T.astype(np.float32)).astype(bf),
        "lp": np.ascontiguousarray(lp.reshape(P, NPAIR * P)).astype(bf),
        "ones32": np.ascontiguousarray(_ones32().reshape(P, NPAIR * C_LOC)).astype(bf),
    }


def build_nc():
    import concourse.bass as bass
    import concourse.tile as tile
    from concourse import bacc, mybir
    from concourse.masks import make_identity
    from contextlib import ExitStack

    f32 = mybir.dt.float32
    bf16 = mybir.dt.bfloat16
    AF = mybir.ActivationFunctionType
    ALU = mybir.AluOpType

    nc = bacc.Bacc(None, target_bir_lowering=False)
    emb_d = nc.dram_tensor("emb", [B, D], bf16, kind="ExternalInput")
    vt_d = nc.dram_tensor("vt", [D, CS], bf16, kind="ExternalInput")
    wt_d = nc.dram_tensor("wt", [D, CS], bf16, kind="ExternalInput")
    lp_d = nc.dram_tensor("lp", [P, NPAIR * P], bf16, kind="ExternalInput")
    o32_d = nc.dram_tensor("ones32", [P, NPAIR * C_LOC], bf16, kind="ExternalInput")
    out_d = nc.dram_tensor("out", [B, C_LOC], f32, kind="ExternalOutput")

    with tile.TileContext(nc) as tc, ExitStack() as ctx:
        sing = ctx.enter_context(tc.tile_pool(name="sing", bufs=1))
        dram = ctx.enter_context(tc.tile_pool(name="dram", bufs=1, space="DRAM"))
        wexp = ctx.enter_context(tc.tile_pool(name="wexp", bufs=4))
        wmask = ctx.enter_context(tc.tile_pool(name="wmask", bufs=3))
        wE = ctx.enter_context(tc.tile_pool(name="wE", bufs=2))
        wF = ctx.enter_context(tc.tile_pool(name="wF", bufs=3))
        wprod = ctx.enter_context(tc.tile_pool(name="wprod", bufs=3))
        small = ctx.enter_context(tc.tile_pool(name="small", bufs=2))
        ps_d = ctx.enter_context(tc.tile_pool(name="ps_d", bufs=2, space="PSUM"))
        ps_t = ctx.enter_context(tc.tile_pool(name="ps_t", bufs=2, space="PSUM"))
        ps_m = ctx.enter_context(tc.tile_pool(name="ps_m", bufs=2, space="PSUM"))
        ps_r = ctx.enter_context(tc.tile_pool(name="ps_r", bufs=1, space="PSUM"))
        ps_o = ctx.enter_context(tc.tile_pool(name="ps_o", bufs=1, space="PSUM"))

        # ---------------- input DMAs (two queues) ----------------
        En = sing.tile([P, NBT, D], bf16)
        nc.sync.dma_start(En[:], emb_d[:].rearrange("(t p) d -> p t d", p=P))
        VT = sing.tile([P, CS], bf16)
        nc.sync.dma_start(VT[:], vt_d[:])
        WT = sing.tile([P, CS], bf16)
        nc.gpsimd.dma_start(WT[:], wt_d[:])
        LP = sing.tile([P, NPAIR, P], bf16)
        nc.gpsimd.dma_start(LP[:], lp_d[:].rearrange("p (q m) -> p q m", q=NPAIR))
        O32 = sing.tile([P, NPAIR, C_LOC], bf16)
        nc.gpsimd.dma_start(O32[:], o32_d[:].rearrange("p (q m) -> p q m", q=NPAIR))

        ident = sing.tile([P, P], bf16)
        make_identity(nc, ident[:])
        sbias = sing.tile([P, 1], f32)
        nc.vector.memset(sbias[:], SC_BIAS)
        lhalf = sing.tile([P, 1], f32)
        nc.vector.memset(lhalf[:], float(np.log(0.5)))
        obias = sing.tile([P, 1], f32)
        nc.vector.memset(obias[:], OUT_BIAS)

        # warm the act table (ln+exp+square+copy+identity in one set)
        tdum = sing.tile([P, 1], f32)
        nc.scalar.activation(tdum[:], sbias[:], AF.Exp)
        nc.scalar.activation(tdum[:], sbias[:], AF.Ln)

        # ---------------- emb norms ----------------
        esq = sing.tile([P, NBT], f32)
        for t in range(NBT):
            j = small.tile([P, D], f32, tag="jact", bufs=2)
            nc.scalar.activation(j[:], En[:, t], AF.Square,
                                 accum_out=esq[:, t : t + 1])
        lesq = sing.tile([P, NBT], f32)
        nc.scalar.activation(lesq[:], esq[:], AF.Ln)
        ine = sing.tile([P, NBT], f32)       # 1/|emb| = exp(-0.5 ln esq)
        nc.scalar.activation(ine[:], lesq[:], AF.Exp, scale=-0.5)
        hine = sing.tile([P, NBT], f32)      # 0.5/|emb|
        nc.scalar.activation(hine[:], lesq[:], AF.Exp, scale=-0.5,
                             bias=lhalf[:])

        # ineT [C_LOC, B]: ine replicated across 32 partitions via DRAM bcast
        scr = dram.tile([B], f32)
        nc.sync.dma_start(scr[:].rearrange("(t p) -> p t", p=P), ine[:])
        scr_bc = bass.AP(
            tensor=scr[:].tensor, offset=scr[:].offset,
            ap=[[0, C_LOC]] + list(scr[:].ap),
        )
        ineT = sing.tile([C_LOC, B], f32)
        nc.sync.dma_start(ineT[:], scr_bc)

        # embT [D, B] bf16 via PE transposes
        embT = sing.tile([P, B], bf16)
        for g in range(2):
            pst = ps_t.tile([P, 4, P], bf16, tag="tr")
            for h in range(4):
                nc.tensor.transpose(pst[:, h], En[:, 4 * g + h], ident[:])
            nc.scalar.copy(embT[:, g * 512 : (g + 1) * 512],
                           pst[:].rearrange("p h x -> p (h x)"))

        # persistent output staging
        osb = sing.tile([P, NBT, C_LOC], f32)

        tiles = {}

        # ---------------- per-tile stages ----------------
        def SA(bt):
            """dot matmuls + exp"""
            bsl = slice(bt * P, (bt + 1) * P)
            exps = wexp.tile([P, CS], bf16, tag="exps")
            for j in range(4):
                js = slice(j * 512, (j + 1) * 512)
                psd = ps_d.tile([P, 512], f32, tag="dot")
                nc.tensor.matmul(psd[:], embT[:, bsl], VT[:, js])
                nc.scalar.activation(exps[:, js], psd[:], AF.Exp,
                                     bias=sbias[:], scale=hine[:, bt : bt + 1])
            tiles[("exps", bt)] = exps

        def SB(bt):
            """top-8 threshold per class + mask"""
            exps = tiles[("exps", bt)]
            m8 = small.tile([P, C_LOC, 8], bf16, tag="m8")
            for c in range(C_LOC):
                nc.vector.max(m8[:, c], exps[:, c * S : (c + 1) * S])
            mask = wmask.tile([P, CS], bf16, tag="mask")
            t8b = m8[:, :, 7:8].to_broadcast([P, C_LOC, S])
            nc.vector.tensor_tensor(
                mask[:].rearrange("p (c s) -> p c s", c=C_LOC),
                exps[:].rearrange("p (c s) -> p c s", c=C_LOC),
                t8b, op=ALU.is_ge)
            tiles[("mask", bt)] = mask

        def SC(bt):
            """masked E + transposes to s-major F"""
            exps = tiles.pop(("exps", bt))
            mask = tiles.pop(("mask", bt))
            E = wE.tile([P, CS], bf16, tag="E")
            nc.vector.tensor_mul(E[:], mask[:], exps[:])
            F = wF.tile([P, CS], bf16, tag="F")
            for g in range(4):
                pst = ps_t.tile([P, 4, P], bf16, tag="tr")
                for h in range(4):
                    q = 4 * g + h
                    nc.tensor.transpose(pst[:, h], E[:, q * P : (q + 1) * P],
                                        ident[:])
                nc.scalar.copy(F[:, g * 512 : (g + 1) * 512],
                               pst[:].rearrange("p h x -> p (h x)"))
            tiles[("F", bt)] = F

        def SD(bt):
            """M = LP^T F -> Msq; dotT = W embT -> prodD"""
            bsl = slice(bt * P, (bt + 1) * P)
            F = tiles[("F", bt)]
            prod2 = wprod.tile([P, NPAIR, 2, P], bf16, tag="p2")
            dsb = wmask.tile([P, NPAIR, P], bf16, tag="dsb")
            for g in range(4):
                psm = ps_m.tile([P, 4, P], f32, tag="mm")
                for h in range(4):
                    q = 4 * g + h
                    nc.tensor.matmul(psm[:, h], LP[:, q],
                                     F[:, q * P : (q + 1) * P])
                nc.scalar.activation(prod2[:, 4 * g : 4 * g + 4, 0, :],
                                     psm[:], AF.Square)
                psw = ps_m.tile([P, 4, P], f32, tag="mm")
                for h in range(4):
                    q = 4 * g + h
                    nc.tensor.matmul(psw[:, h], WT[:, q * P : (q + 1) * P],
                                     embT[:, bsl])
                nc.scalar.copy(dsb[:, 4 * g : 4 * g + 4], psw[:])
            nc.gpsimd.tensor_mul(
                prod2[:, :, 1, :], dsb[:],
                F[:].rearrange("p (q x) -> p q x", q=NPAIR))
            tiles[("p2", bt)] = prod2

        def SE(bt):
            """segmented s-sums via accumulating block-ones matmuls"""
            prod2 = tiles.pop(("p2", bt))
            psr = ps_r.tile([C_LOC, 2, P], f32, tag="red")
            for q in range(NPAIR):
                nc.tensor.matmul(psr[:], O32[:, q], prod2[:, q],
                                 start=(q == 0), stop=(q == NPAIR - 1))
            tiles[("red", bt)] = psr

        npd = sing.tile([C_LOC, NBT, 2, P], f32)

        def SF(bt):
            """stage the per-tile reduction results; tail is batched at end"""
            psr = tiles.pop(("red", bt))
            nc.scalar.copy(npd[:, bt], psr[:])

        # ---------------- software-pipelined loop ----------------
        for r in range(NBT + 5):
            if 0 <= r - 5:
                SF(r - 5)
            if 0 <= r - 4 < NBT:
                SE(r - 4)
            if 0 <= r - 3 < NBT:
                SD(r - 3)
            if 0 <= r - 2 < NBT:
                SC(r - 2)
            if 0 <= r - 1 < NBT:
                SB(r - 1)
            if r < NBT:
                SA(r)

        # ---------------- batched tail ----------------
        # cos2 = d2' * rsqrt(np2), rsqrt via exp(-0.5 ln); out = 5*cos2+5+1e-7
        lnp = sing.tile([C_LOC, NBT, P], f32)
        nc.scalar.activation(lnp[:], npd[:, :, 0, :], AF.Ln)
        rnp = sing.tile([C_LOC, NBT, P], f32)
        nc.scalar.activation(rnp[:], lnp[:], AF.Exp, scale=-0.5)
        c1 = sing.tile([C_LOC, NBT, P], bf16)
        nc.vector.tensor_mul(c1[:], npd[:, :, 1, :], rnp[:])
        c2 = sing.tile([C_LOC, NBT, P], bf16)
        nc.vector.tensor_mul(c2[:], c1[:],
                             ineT[:].rearrange("c (t p) -> c t p", p=P))
        for bt in range(NBT):
            pso = ps_o.tile([P, C_LOC], bf16, tag="out")
            nc.tensor.transpose(pso[:], c2[:, bt], ident[:C_LOC, :C_LOC])
            nc.scalar.activation(osb[:, bt], pso[:], AF.Identity,
                                 bias=obias[:], scale=OUT_SCALE)

        nc.sync.dma_start(out_d[:].rearrange("(t p) c -> p t c", p=P), osb[:])

    nc.compile()
    return nc


def _get_nc():
    if "nc" not in _CACHE:
        _CACHE["nc"] = build_nc()
    return _CACHE["nc"]


def kernel(emb: np.ndarray, weight: np.ndarray) -> np.ndarray:
    from concourse.bass_utils import run_bass_kernel_spmd

    emb = np.ascontiguousarray(np.asarray(emb, dtype=np.float32))
    weight = np.ascontiguousarray(np.asarray(weight, dtype=np.float32))
    assert emb.shape == (B, D) and weight.shape == (C, S, D)

    nc = _get_nc()
    in_maps = [core_inputs(emb, weight, i) for i in range(NCORES)]
    res = run_bass_kernel_spmd(nc, in_maps, core_ids=list(range(NCORES)))
    return np.concatenate(
        [res.results[i]["out"] for i in range(NCORES)], axis=1
    ).astype(np.float32)


# revision 15
# speedup vs baseline: 1.2430x; 1.2430x over previous
"""Trainium2 Bass kernel for the moe_routing classifier problem (v2).

Computation (per batch row b, class c):
  score[b,c,s] = (1 + cos(emb[b], W[c,s]))/2 + 1e-8     (S=64 sub-prototypes)
  top-8 over s, softmax weights w, protos = sum_k w_k * W[c, idx_k]
  out[b,c]     = ((1 + cos(protos, emb[b]))/2 + 1e-8) / 0.1

Algebra (Z = unnormalized softmax sum cancels in the cosine ratio):
  E[b,cs]   = exp(score) masked to the top-8 entries   (threshold mask)
  d2'[b,c]  = sum_s E * (W_s . emb_b/|emb_b|)          (prodD in s-major space)
  np2[b,c]  = |L_c^T E_c^T|^2 = E^T G E                (G = W W^T = L L^T, host chol)
  out       = 5 * d2' / sqrt(np2) + 5 + 1e-7           (1/sqrt via exp(-.5 ln))

Host prep (per core, weights-only): V^T (rows normalized), W^T, block-diag
pair Cholesky factors LP, a block-ones reduction matrix; all bf16.

Device per batch tile (128 rows):
  PE : dot = embT^T V       (b-major, for scores)
       dotT = W embT        (s-major, raw dots for d2')
       F = E^T (16 transposes), M = LP^T F, segmented s-sums of
       [M^2 | F*dotT] via 16 accumulating block-ones matmuls -> psum [32,2,128]
  ACT: exps = Exp(dot*hine+bias) bf16; F/Msq copies; tail ln/exp/out-copy
  DVE: 32x MAX8 (8th-largest per class), E = mask*exps (bf16 2x), tail muls
  POOL: mask = exps >= t8 (broadcast threshold), prodD = dotT * F

Engines never touch Sqrt: all ACT funcs (Exp/Ln/Square/Copy/Identity) live in
the natural_log_exp_and_others table -> zero table reloads.

Sharding: classes split across 8 cores (32 each); emb replicated.
"""

import numpy as np

B, D, C, S = 1024, 128, 256, 64
NCORES = 8
C_LOC = C // NCORES        # 32 classes per core
CS = C_LOC * S             # 2048 anchor rows per core
P = 128                    # partitions
NBT = B // P               # 8 batch tiles
NPAIR = C_LOC // 2         # 16 class pairs (128 anchor rows each)
EPS = 1e-8
SC_BIAS = 0.5 + EPS        # score = 0.5*cos + SC_BIAS
OUT_SCALE = 5.0            # ((1+x)/2 + 1e-8) / 0.1 = 5x + 5 + 1e-7
OUT_BIAS = 5.0 + 1e-7

_CACHE = {}


def _ones32() -> np.ndarray:
    """Block-ones stationary [128, NPAIR, C_LOC]: chunk q reduces partitions
    0:64 -> class 2q, 64:128 -> class 2q+1; all other columns zero so the 16
    matmuls can accumulate into one [C_LOC, ...] psum region."""
    o = np.zeros((P, NPAIR, C_LOC), np.float32)
    for q in range(NPAIR):
        o[0:64, q, 2 * q] = 1.0
        o[64:128, q, 2 * q + 1] = 1.0
    return o


def core_inputs(emb: np.ndarray, weight: np.ndarray, i: int) -> dict:
    """Host-side prep for core i: bf16 cast, V/W transposes, pair-packed
    Cholesky factors of the per-class Gram matrices."""
    import ml_dtypes

    bf = ml_dtypes.bfloat16
    Wc = np.ascontiguousarray(weight[i * C_LOC : (i + 1) * C_LOC]).astype(
        np.float64
    )                                              # [32, 64, 128]
    G = np.einsum("csd,ctd->cst", Wc, Wc)          # [32, 64, 64]
    jit = 1e-9 * np.einsum("css->c", G) / S
    G += jit[:, None, None] * np.eye(S)
    L = np.linalg.cholesky(G)                      # lower: G = L L^T
    lp = np.zeros((P, NPAIR, P), np.float32)
    for q in range(NPAIR):
        lp[0:S, q, 0:S] = L[2 * q]
        lp[S:P, q, S:P] = L[2 * q + 1]
    W2 = Wc.reshape(CS, D)
    nw = np.maximum(np.linalg.norm(W2, axis=1), EPS)
    V2 = W2 / nw[:, None]
    return {
        "emb": emb.astype(bf),
        "vt": np.ascontiguousarray(V2.T.astype(np.float32)).astype(bf),
        "wt": np.ascontiguousarray(W2.T.astype(np.float32)).astype(bf),
        "lp": np.ascontiguousarray(lp.reshape(P, NPAIR * P)).astype(bf),
        "ones32": np.ascontiguousarray(_ones32().reshape(P, NPAIR * C_LOC)).astype(bf),
    }


def build_nc():
    import concourse.bass as bass
    import concourse.tile as tile
    from concourse import bacc, mybir
    from concourse.masks import make_identity
    from contextlib import ExitStack

    f32 = mybir.dt.float32
    bf16 = mybir.dt.bfloat16
    AF = mybir.ActivationFunctionType
    ALU = mybir.AluOpType

    nc = bacc.Bacc(None, target_bir_lowering=False)
    emb_d = nc.dram_tensor("emb", [B, D], bf16, kind="ExternalInput")
    vt_d = nc.dram_tensor("vt", [D, CS], bf16, kind="ExternalInput")
    wt_d = nc.dram_tensor("wt", [D, CS], bf16, kind="ExternalInput")
    lp_d = nc.dram_tensor("lp", [P, NPAIR * P], bf16, kind="ExternalInput")
    o32_d = nc.dram_tensor("ones32", [P, NPAIR * C_LOC], bf16, kind="ExternalInput")
    out_d = nc.dram_tensor("out", [B, C_LOC], f32, kind="ExternalOutput")

    with tile.TileContext(nc) as tc, ExitStack() as ctx:
        sing = ctx.enter_context(tc.tile_pool(name="sing", bufs=1))
        dram = ctx.enter_context(tc.tile_pool(name="dram", bufs=1, space="DRAM"))
        wexp = ctx.enter_context(tc.tile_pool(name="wexp", bufs=4))
        wmask = ctx.enter_context(tc.tile_pool(name="wmask", bufs=3))
        wE = ctx.enter_context(tc.tile_pool(name="wE", bufs=2))
        wF = ctx.enter_context(tc.tile_pool(name="wF", bufs=3))
        wprod = ctx.enter_context(tc.tile_pool(name="wprod", bufs=3))
        small = ctx.enter_context(tc.tile_pool(name="small", bufs=2))
        ps_d = ctx.enter_context(tc.tile_pool(name="ps_d", bufs=2, space="PSUM"))
        ps_t = ctx.enter_context(tc.tile_pool(name="ps_t", bufs=2, space="PSUM"))
        ps_m = ctx.enter_context(tc.tile_pool(name="ps_m", bufs=2, space="PSUM"))
        ps_r = ctx.enter_context(tc.tile_pool(name="ps_r", bufs=1, space="PSUM"))
        ps_o = ctx.enter_context(tc.tile_pool(name="ps_o", bufs=1, space="PSUM"))

        # ---------------- input DMAs (two queues) ----------------
        En = sing.tile([P, NBT, D], bf16)
        nc.sync.dma_start(En[:], emb_d[:].rearrange("(t p) d -> p t d", p=P))
        VT = sing.tile([P, CS], bf16)
        for j in range(4):
            js = slice(j * 512, (j + 1) * 512)
            nc.sync.dma_start(VT[:, js], vt_d[:, js])
        WT = sing.tile([P, CS], bf16)
        nc.gpsimd.dma_start(WT[:], wt_d[:])
        LP = sing.tile([P, NPAIR, P], bf16)
        nc.gpsimd.dma_start(LP[:], lp_d[:].rearrange("p (q m) -> p q m", q=NPAIR))
        O32 = sing.tile([P, NPAIR, C_LOC], bf16)
        nc.gpsimd.dma_start(O32[:], o32_d[:].rearrange("p (q m) -> p q m", q=NPAIR))

        ident = sing.tile([P, P], bf16)
        make_identity(nc, ident[:])
        sbias = sing.tile([P, 1], f32)
        nc.vector.memset(sbias[:], SC_BIAS)
        lhalf = sing.tile([P, 1], f32)
        nc.vector.memset(lhalf[:], float(np.log(0.5)))
        obias = sing.tile([P, 1], f32)
        nc.vector.memset(obias[:], OUT_BIAS)

        # warm the act table (ln+exp+square+copy+identity in one set)
        tdum = sing.tile([P, 1], f32)
        nc.scalar.activation(tdum[:], sbias[:], AF.Exp)
        nc.scalar.activation(tdum[:], sbias[:], AF.Ln)

        # ---------------- emb norms ----------------
        esq = sing.tile([P, NBT], f32)
        for t in range(NBT):
            j = small.tile([P, D], f32, tag="jact", bufs=2)
            nc.scalar.activation(j[:], En[:, t], AF.Square,
                                 accum_out=esq[:, t : t + 1])
        lesq = sing.tile([P, NBT], f32)
        nc.scalar.activation(lesq[:], esq[:], AF.Ln)
        ine = sing.tile([P, NBT], f32)       # 1/|emb| = exp(-0.5 ln esq)
        nc.scalar.activation(ine[:], lesq[:], AF.Exp, scale=-0.5)
        hine = sing.tile([P, NBT], f32)      # 0.5/|emb|
        nc.scalar.activation(hine[:], lesq[:], AF.Exp, scale=-0.5,
                             bias=lhalf[:])

        # ineT [C_LOC, B]: ine replicated across 32 partitions via DRAM bcast
        scr = dram.tile([B], f32)
        nc.sync.dma_start(scr[:].rearrange("(t p) -> p t", p=P), ine[:])
        scr_bc = bass.AP(
            tensor=scr[:].tensor, offset=scr[:].offset,
            ap=[[0, C_LOC]] + list(scr[:].ap),
        )
        ineT = sing.tile([C_LOC, B], f32)
        nc.sync.dma_start(ineT[:], scr_bc)

        # embT [D, B] bf16 via PE transposes
        embT = sing.tile([P, B], bf16)
        for g in range(2):
            pst = ps_t.tile([P, 4, P], bf16, tag="tr")
            for h in range(4):
                nc.tensor.transpose(pst[:, h], En[:, 4 * g + h], ident[:])
            nc.scalar.copy(embT[:, g * 512 : (g + 1) * 512],
                           pst[:].rearrange("p h x -> p (h x)"))

        # persistent output staging
        osb = sing.tile([P, NBT, C_LOC], f32)

        tiles = {}

        # ---------------- per-tile stages ----------------
        def SA(bt):
            """dot matmuls + exp"""
            bsl = slice(bt * P, (bt + 1) * P)
            exps = wexp.tile([P, CS], bf16, tag="exps")
            for j in range(4):
                js = slice(j * 512, (j + 1) * 512)
                psd = ps_d.tile([P, 512], f32, tag="dot")
                nc.tensor.matmul(psd[:], embT[:, bsl], VT[:, js])
                nc.scalar.activation(exps[:, js], psd[:], AF.Exp,
                                     bias=sbias[:], scale=hine[:, bt : bt + 1])
            tiles[("exps", bt)] = exps

        def SB(bt):
            """top-8 threshold per class + mask"""
            exps = tiles[("exps", bt)]
            m8 = small.tile([P, C_LOC, 8], bf16, tag="m8")
            for c in range(C_LOC):
                nc.vector.max(m8[:, c], exps[:, c * S : (c + 1) * S])
            mask = wmask.tile([P, CS], bf16, tag="mask")
            t8b = m8[:, :, 7:8].to_broadcast([P, C_LOC, S])
            nc.vector.tensor_tensor(
                mask[:].rearrange("p (c s) -> p c s", c=C_LOC),
                exps[:].rearrange("p (c s) -> p c s", c=C_LOC),
                t8b, op=ALU.is_ge)
            tiles[("mask", bt)] = mask

        def SC(bt):
            """masked E + transposes to s-major F"""
            exps = tiles.pop(("exps", bt))
            mask = tiles.pop(("mask", bt))
            E = wE.tile([P, CS], bf16, tag="E")
            nc.vector.tensor_mul(E[:], mask[:], exps[:])
            F = wF.tile([P, CS], bf16, tag="F")
            for g in range(4):
                pst = ps_t.tile([P, 4, P], bf16, tag="tr")
                for h in range(4):
                    q = 4 * g + h
                    nc.tensor.transpose(pst[:, h], E[:, q * P : (q + 1) * P],
                                        ident[:])
                nc.scalar.copy(F[:, g * 512 : (g + 1) * 512],
                               pst[:].rearrange("p h x -> p (h x)"))
            tiles[("F", bt)] = F

        def SD(bt):
            """M = LP^T F -> Msq; dotT = W embT -> prodD"""
            bsl = slice(bt * P, (bt + 1) * P)
            F = tiles[("F", bt)]
            prod2 = wprod.tile([P, NPAIR, 2, P], bf16, tag="p2")
            dsb = wmask.tile([P, NPAIR, P], bf16, tag="dsb")
            for g in range(4):
                psm = ps_m.tile([P, 4, P], f32, tag="mm")
                for h in range(4):
                    q = 4 * g + h
                    nc.tensor.matmul(psm[:, h], LP[:, q],
                                     F[:, q * P : (q + 1) * P])
                nc.scalar.activation(prod2[:, 4 * g : 4 * g + 4, 0, :],
                                     psm[:], AF.Square)
                psw = ps_m.tile([P, 4, P], f32, tag="mm")
                for h in range(4):
                    q = 4 * g + h
                    nc.tensor.matmul(psw[:, h], WT[:, q * P : (q + 1) * P],
                                     embT[:, bsl])
                nc.scalar.copy(dsb[:, 4 * g : 4 * g + 4], psw[:])
                nc.gpsimd.tensor_mul(
                    prod2[:, 4 * g : 4 * g + 4, 1, :],
                    dsb[:, 4 * g : 4 * g + 4],
                    F[:, g * 512 : (g + 1) * 512].rearrange(
                        "p (h x) -> p h x", h=4))
            tiles[("p2", bt)] = prod2

        def SE(bt):
            """segmented s-sums via accumulating block-ones matmuls"""
            prod2 = tiles.pop(("p2", bt))
            psr = ps_r.tile([C_LOC, 2, P], f32, tag="red")
            for q in range(NPAIR):
                nc.tensor.matmul(psr[:], O32[:, q], prod2[:, q],
                                 start=(q == 0), stop=(q == NPAIR - 1))
            tiles[("red", bt)] = psr

        npd = sing.tile([C_LOC, NBT, 2, P], f32)

        def SF(bt):
            """stage the per-tile reduction results; tail is batched at end"""
            psr = tiles.pop(("red", bt))
            nc.scalar.copy(npd[:, bt], psr[:])

        # batched tail halves: cos2 = d2' * rsqrt(np2) (rsqrt via exp(-.5 ln))
        lnp = sing.tile([C_LOC, NBT, P], f32)
        rnp = sing.tile([C_LOC, NBT, P], f32)
        c1 = sing.tile([C_LOC, NBT, P], bf16)
        c2 = sing.tile([C_LOC, NBT, P], bf16)
        ineT3 = ineT[:].rearrange("c (t p) -> c t p", p=P)
        outv = out_d[:].rearrange("(t p) c -> p t c", p=P)

        def tail_half(t0, t1):
            ts = slice(t0, t1)
            nc.scalar.activation(lnp[:, ts], npd[:, ts, 0, :], AF.Ln)
            nc.scalar.activation(rnp[:, ts], lnp[:, ts], AF.Exp, scale=-0.5)
            nc.vector.tensor_mul(c1[:, ts], npd[:, ts, 1, :], rnp[:, ts])
            nc.vector.tensor_mul(c2[:, ts], c1[:, ts], ineT3[:, ts])
            for bt in range(t0, t1):
                pso = ps_o.tile([P, C_LOC], bf16, tag="out")
                nc.tensor.transpose(pso[:], c2[:, bt], ident[:C_LOC, :C_LOC])
                nc.scalar.activation(osb[:, bt], pso[:], AF.Identity,
                                     bias=obias[:], scale=OUT_SCALE)
            nc.sync.dma_start(outv[:, ts], osb[:, ts])

        # ---------------- software-pipelined loop ----------------
        for r in range(NBT + 5):
            if 0 <= r - 5:
                SF(r - 5)
            if r - 5 == NBT // 2 - 1:
                tail_half(0, NBT // 2)
            if 0 <= r - 4 < NBT:
                SE(r - 4)
            if 0 <= r - 3 < NBT:
                SD(r - 3)
            if 0 <= r - 2 < NBT:
                SC(r - 2)
            if 0 <= r - 1 < NBT:
                SB(r - 1)
            if r < NBT:
                SA(r)

        tail_half(NBT // 2, NBT)

    nc.compile()
    return nc


def _get_nc():
    if "nc" not in _CACHE:
        _CACHE["nc"] = build_nc()
    return _CACHE["nc"]


def kernel(emb: np.ndarray, weight: np.ndarray) -> np.ndarray:
    from concourse.bass_utils import run_bass_kernel_spmd

    emb = np.ascontiguousarray(np.asarray(emb, dtype=np.float32))
    weight = np.ascontiguousarray(np.asarray(weight, dtype=np.float32))
    assert emb.shape == (B, D) and weight.shape == (C, S, D)

    nc = _get_nc()
    in_maps = [core_inputs(emb, weight, i) for i in range(NCORES)]
    res = run_bass_kernel_spmd(nc, in_maps, core_ids=list(range(NCORES)))
    return np.concatenate(
        [res.results[i]["out"] for i in range(NCORES)], axis=1
    ).astype(np.float32)
